# revision 1
# baseline (speedup 1.0000x reference)
"""BiGRU encoder kernel for 8 Trainium2 NeuronCores.

Strategy:
  - Reformulate the per-sample ragged windows as masked GRUs over FIXED
    position ranges: forward runs positions 0..7 ascending, backward runs
    positions 14..7 descending.  A sample with forward length lf only starts
    updating at position 8-lf; before that its hidden state must stay 0.
    That is enforced exactly by adding +BIG to the z-gate pre-activation for
    pre-start steps (z==1.0 => h' = n + z*(h-n) = n + (0-n) = 0 exactly).
  - Sort samples by window_len, deal them round-robin to the 8 cores (data
    parallel, near-identical length distribution per core).  Per core, two
    batch tiles of 512 samples; each GRU step runs only on the suffix of
    samples that are long enough to need it (suffix clamped to >=256 so
    float32r matmuls stay at full rate; over-included samples are exact via
    the z-mask and h-prefix memsets).
  - Everything on-device is computed in transposed (feature-major) layout:
    features on SBUF partitions, samples on the free dim, so the recurrence
    needs no runtime transposes.  Weights are transposed host-side.
  - Matmuls run as float32r (full-rate fp32 mode of the PE array).
"""

import os
from contextlib import ExitStack

import numpy as np

import concourse.bacc as bacc
import concourse.tile as tile
from concourse import mybir
from concourse.bass_utils import run_bass_kernel_spmd
from concourse.masks import make_identity

NCORES = 8
B, T, D, H = 8192, 15, 512, 512
G = 3 * H  # gate rows (r, z, n)
BIG = 40.0
S = 512  # samples per batch tile
F32 = mybir.dt.float32
DT_MM = mybir.dt.float32 if os.environ.get("GRU_DT") == "f32" else mybir.dt.float32r
H_ENGINE = os.environ.get("GRU_HUPD", "vector")  # engine for h-update chain

ACT = mybir.ActivationFunctionType
ALU = mybir.AluOpType

_PROGRAM_CACHE = {}
LAST_RESULT = None


def _build_program(sched):
    """sched: per tile, (f_steps, b_steps); each step = (width, masked)."""
    ntiles = len(sched)
    Bc = S * ntiles
    nc = bacc.Bacc("TRN2", target_bir_lowering=False, debug=False,
                   num_devices=NCORES)

    xT_d = nc.dram_tensor("xT", [T, D, Bc], DT_MM, kind="ExternalInput")
    wf_d = nc.dram_tensor("wf", [D + H, G], DT_MM, kind="ExternalInput")
    wb_d = nc.dram_tensor("wb", [D + H, G], DT_MM, kind="ExternalInput")
    w1_d = nc.dram_tensor("w1", [2 * H, H], DT_MM, kind="ExternalInput")
    w2_d = nc.dram_tensor("w2", [H, H], DT_MM, kind="ExternalInput")
    bias_d = nc.dram_tensor("bias", [40, 128], F32, kind="ExternalInput")
    mf_d = nc.dram_tensor("maskzf", [8, Bc], F32, kind="ExternalInput")
    mb_d = nc.dram_tensor("maskzb", [8, Bc], F32, kind="ExternalInput")
    y_d = nc.dram_tensor("y", [Bc, H], F32, kind="ExternalOutput")

    with tile.TileContext(nc) as tc, ExitStack() as ctx:
        const = ctx.enter_context(tc.tile_pool(name="const", bufs=1))
        wpool = ctx.enter_context(tc.tile_pool(name="w", bufs=2))
        xpool = ctx.enter_context(tc.tile_pool(name="x", bufs=2))
        hpool = ctx.enter_context(tc.tile_pool(name="h", bufs=2))
        hfin = ctx.enter_context(tc.tile_pool(name="hfin", bufs=4))
        gpool = ctx.enter_context(tc.tile_pool(name="g", bufs=5))
        mpool = ctx.enter_context(tc.tile_pool(name="m", bufs=1))
        opool = ctx.enter_context(tc.tile_pool(name="o", bufs=4))
        rzps = ctx.enter_context(tc.tile_pool(name="rz", bufs=4, space="PSUM"))
        xpps = ctx.enter_context(tc.tile_pool(name="xp", bufs=2, space="PSUM"))
        ghps = ctx.enter_context(tc.tile_pool(name="gh", bufs=2, space="PSUM"))

        # Weights [128, kchunk, gate-cols]; kchunks 0-3 input dims, 4-7 hidden
        # dims.  wf/wb/w1 time-share a 2-slot pool (one tag); per-kchunk DMAs
        # so the first matmuls start as soon as chunk 0 lands.
        def load_w(dram, kchunks, cols, name, pool=None, sync_chunks=()):
            t_ = (pool or wpool).tile([128, kchunks, cols], DT_MM,
                                      tag="w" if pool is None else "const",
                                      name=name)
            src = dram.rearrange("(c k) g -> k c g", k=128)
            for c in range(kchunks):
                eng = nc.sync if c in sync_chunks else nc.scalar
                eng.dma_start(t_[:, c, :], src[:, c, :])
            return t_

        wf = load_w(wf_d, 8, G, "wf", sync_chunks=(0, 1, 2, 3))
        wb = load_w(wb_d, 8, G, "wb")
        w2 = load_w(w2_d, 4, H, "w2", pool=const)
        bt = const.tile([128, 40], F32)
        nc.gpsimd.dma_start(bt[:], bias_d.rearrange("n p -> p n"))
        ident = const.tile([128, 128], F32)
        make_identity(nc, ident[:])

        heng = nc.gpsimd if H_ENGINE == "gpsimd" else nc.vector

        def emit_dir(s0, steps, w, mask_d, bb, pos_fn):
            """One GRU direction over one batch tile; returns final h tile."""
            nsteps = len(steps)
            h_prev = None
            for j, (width, masked) in enumerate(steps):
                first = j == 0
                p_abs = pos_fn(j)
                so = S - width  # suffix offset within the tile
                a0 = s0 + so
                xt = xpool.tile([128, 4, S], DT_MM, tag="x", name="xt")
                nc.sync.dma_start(
                    xt[:, :, :width],
                    xT_d[p_abs].rearrange("(c k) s -> k c s", k=128)[:, :, a0:s0 + S],
                )
                mt = None
                if masked:
                    mt = mpool.tile([128, S], F32, tag="m", name="mt")
                    nc.gpsimd.dma_start(
                        mt[:, :width],
                        mask_d[8 - nsteps + j, a0:s0 + S].partition_broadcast(128),
                    )
                h_next = (hfin if j == nsteps - 1 else hpool).tile(
                    [128, 4, S], DT_MM, tag="hf" if j == nsteps - 1 else "h",
                    name="h")
                if j + 1 < nsteps:
                    nso = S - steps[j + 1][0]  # next step's suffix offset
                    if nso < so:
                        nc.gpsimd.memset(h_next[:, :, nso:so].bitcast(F32), 0.0)

                rps, zps, xpns, ghns = [], [], [], []
                for i in range(4):
                    # separate PSUM tiles per accumulation group: start=True
                    # clears the whole bank, so groups must not share one
                    r_ps = rzps.tile([128, width], F32, tag="rz", name=f"rps{i}")
                    z_ps = rzps.tile([128, width], F32, tag="rz", name=f"zps{i}")
                    xpn = xpps.tile([128, width], F32, tag="xp", name=f"xpn{i}")
                    rps.append(r_ps)
                    zps.append(z_ps)
                    xpns.append(xpn)
                    for k in range(4):
                        st = k == 0
                        sp_rz = first and k == 3
                        xk = xt[:, k, :width]
                        nc.tensor.matmul(r_ps[:], w[:, k, i * 128:(i + 1) * 128],
                                         xk, start=st, stop=sp_rz)
                        nc.tensor.matmul(z_ps[:],
                                         w[:, k, H + i * 128:H + (i + 1) * 128],
                                         xk, start=st, stop=sp_rz)
                        nc.tensor.matmul(xpn[:],
                                         w[:, k, 2 * H + i * 128:2 * H + (i + 1) * 128],
                                         xk, start=st, stop=k == 3)
                if not first:
                    for i in range(4):
                        ghn = ghps.tile([128, width], F32, tag="gh", name=f"ghn{i}")
                        ghns.append(ghn)
                        for k in range(4):
                            hk = h_prev[:, k, so:]
                            nc.tensor.matmul(rps[i][:],
                                             w[:, 4 + k, i * 128:(i + 1) * 128],
                                             hk, start=False, stop=k == 3)
                            nc.tensor.matmul(zps[i][:],
                                             w[:, 4 + k, H + i * 128:H + (i + 1) * 128],
                                             hk, start=False, stop=k == 3)
                            nc.tensor.matmul(ghn[:],
                                             w[:, 4 + k, 2 * H + i * 128:2 * H + (i + 1) * 128],
                                             hk, start=k == 0, stop=k == 3)

                for i in range(4):
                    xpn = xpns[i]
                    r = gpool.tile([128, width], F32, tag="g", name="r")
                    nc.scalar.activation(r[:], rps[i][:], ACT.Sigmoid,
                                         bias=bt[:, bb + i:bb + i + 1])
                    if masked:
                        zin = gpool.tile([128, width], F32, tag="g", name="zin")
                        nc.vector.tensor_add(zin[:], zps[i][:], mt[:, :width])
                        zsrc = zin[:]
                    else:
                        zsrc = zps[i][:]
                    z = gpool.tile([128, width], F32, tag="g", name="z")
                    nc.scalar.activation(z[:], zsrc, ACT.Sigmoid,
                                         bias=bt[:, bb + 4 + i:bb + 5 + i])
                    tt = gpool.tile([128, width], F32, tag="g", name="tt")
                    if first:
                        nc.vector.tensor_scalar_mul(tt[:], r[:],
                                                    bt[:, bb + 8 + i:bb + 9 + i])
                    else:
                        nc.vector.scalar_tensor_tensor(
                            tt[:], ghns[i][:], bt[:, bb + 8 + i:bb + 9 + i], r[:],
                            op0=ALU.add, op1=ALU.mult)
                    ss = gpool.tile([128, width], F32, tag="g", name="ss")
                    nc.vector.tensor_add(ss[:], tt[:], xpn[:])
                    n = gpool.tile([128, width], F32, tag="g", name="n")
                    nc.scalar.activation(n[:], ss[:], ACT.Tanh,
                                         bias=bt[:, bb + 12 + i:bb + 13 + i])
                    ho = h_next[:, i, so:]
                    if first:
                        e = gpool.tile([128, width], F32, tag="g", name="e")
                        heng.tensor_mul(e[:], z[:], n[:])
                        heng.tensor_sub(ho, n[:], e[:])
                    else:
                        dd = gpool.tile([128, width], F32, tag="g", name="dd")
                        heng.tensor_sub(dd[:], h_prev[:, i, so:], n[:])
                        e = gpool.tile([128, width], F32, tag="g", name="e")
                        heng.tensor_mul(e[:], z[:], dd[:])
                        heng.tensor_add(ho, n[:], e[:])
                h_prev = h_next
            return h_prev

        hfs = []
        for t in range(ntiles):
            nf = len(sched[t][0])
            hfs.append(emit_dir(t * S, sched[t][0], wf, mf_d, 0,
                                lambda j, nf=nf: 8 - nf + j))
        w1 = load_w(w1_d, 8, H, "w1")

        def emit_mlp(t, hf, hb):
            hid = []
            for i in range(4):
                ps = xpps.tile([128, S], F32, tag="xp", name="mps")
                for k in range(8):
                    src = hf if k < 4 else hb
                    nc.tensor.matmul(ps[:], w1[:, k, i * 128:(i + 1) * 128],
                                     src[:, k % 4, :], start=k == 0, stop=k == 7)
                h32 = gpool.tile([128, S], F32, tag="g", name="h32")
                nc.scalar.activation(h32[:], ps[:], ACT.Relu,
                                     bias=bt[:, 32 + i:33 + i])
                hr = gpool.tile([128, S], DT_MM, tag="g", name="hr")
                nc.vector.tensor_copy(hr[:], h32[:])
                hid.append(hr)
            onats = []
            for gidx in range(S // 128):
                onat = opool.tile([128, H], F32, tag="o", name=f"onat{gidx}")
                onats.append(onat)
            for i in range(4):
                ps = xpps.tile([128, S], F32, tag="xp", name="ops")
                for k in range(4):
                    nc.tensor.matmul(ps[:], w2[:, k, i * 128:(i + 1) * 128],
                                     hid[k][:], start=k == 0, stop=k == 3)
                o32 = gpool.tile([128, S], F32, tag="g", name="o32")
                nc.vector.tensor_scalar_add(o32[:], ps[:], bt[:, 36 + i:37 + i])
                for gidx in range(S // 128):
                    tp = ghps.tile([128, 128], F32, tag="gh", name="tp")
                    nc.tensor.transpose(tp[:], o32[:, gidx * 128:(gidx + 1) * 128],
                                        ident[:])
                    nc.vector.tensor_copy(onats[gidx][:, i * 128:(i + 1) * 128],
                                          tp[:])
            for gidx in range(S // 128):
                r0 = t * S + gidx * 128
                nc.sync.dma_start(y_d[r0:r0 + 128, :], onats[gidx][:])

        for t in range(ntiles):
            nb = len(sched[t][1])
            hb = emit_dir(t * S, sched[t][1], wb, mb_d, 16,
                          lambda j, nb=nb: 6 + nb - j)
            emit_mlp(t, hfs[t], hb)

    nc.compile()
    return nc


def kernel(padded_window, window_len, Wih_f, Whh_f, bih_f, bhh_f,
           Wih_b, Whh_b, bih_b, bhh_b, W1, b1, W2, b2):
    wl = np.asarray(window_len)
    lf = (wl - 1) // 2 + 1
    lb = wl // 2 + 1
    order = np.argsort(wl, kind="stable")

    Bc = B // NCORES
    ntiles = Bc // S
    # per-core sorted lengths: row k = per-core rank k, column = core
    lf_pc = lf[order].reshape(-1, NCORES)
    lb_pc = lb[order].reshape(-1, NCORES)

    def dir_steps(lens_pc, t):
        seg = lens_pc[t * S:(t + 1) * S]  # [S, NCORES]
        n = int(seg.max())
        steps = []
        for j in range(n):
            need = n - j
            cnt = (seg >= need).sum(axis=0)  # samples needing this step, per core
            w = int(min(S, max(256, -(-int(cnt.max()) // 64) * 64)))
            masked = bool(cnt.min() < w)
            steps.append((w, masked))
        return tuple(steps)

    sched = tuple((dir_steps(lf_pc, t), dir_steps(lb_pc, t))
                  for t in range(ntiles))

    if sched not in _PROGRAM_CACHE:
        _PROGRAM_CACHE[sched] = _build_program(sched)
    nc = _PROGRAM_CACHE[sched]

    f32 = np.float32
    wf = np.concatenate([Wih_f.T, Whh_f.T], 0).astype(f32)
    wb = np.concatenate([Wih_b.T, Whh_b.T], 0).astype(f32)
    w1 = np.ascontiguousarray(W1.T, dtype=f32)
    w2 = np.ascontiguousarray(W2.T, dtype=f32)

    def chunks(v):  # [512] -> [4, 128]
        return np.asarray(v, f32).reshape(4, 128)

    bias = np.concatenate([
        chunks((bih_f + bhh_f)[:H]), chunks((bih_f + bhh_f)[H:2 * H]),
        chunks(bhh_f[2 * H:]), chunks(bih_f[2 * H:]),
        chunks((bih_b + bhh_b)[:H]), chunks((bih_b + bhh_b)[H:2 * H]),
        chunks(bhh_b[2 * H:]), chunks(bih_b[2 * H:]),
        chunks(b1), chunks(b2),
    ], 0)  # [40, 128]

    pw = np.asarray(padded_window, f32)
    in_maps = []
    p8 = np.arange(8)
    for c in range(NCORES):
        idx = order[c::NCORES]
        xT = np.ascontiguousarray(pw[idx].transpose(1, 2, 0))  # [15, 512, Bc]
        mzf = (BIG * (p8[:, None] < (8 - lf[idx])[None, :])).astype(f32)
        mzb = (BIG * (p8[:, None] < (8 - lb[idx])[None, :])).astype(f32)
        in_maps.append({
            "xT": xT, "wf": wf, "wb": wb, "w1": w1, "w2": w2,
            "bias": bias, "maskzf": mzf, "maskzb": mzb,
        })

    trace = bool(os.environ.get("GRU_TRACE"))
    kw = {}
    if os.environ.get("GRU_TMPDIR"):
        kw["tmpdir"] = os.environ["GRU_TMPDIR"]
    res = run_bass_kernel_spmd(nc, in_maps, core_ids=list(range(NCORES)),
                               trace=trace, **kw)
    global LAST_RESULT
    LAST_RESULT = res
    out = np.empty((B, H), f32)
    for c in range(NCORES):
        out[order[c::NCORES]] = res.results[c]["y"]
    return out



# revision 5
# speedup vs baseline: 1.2420x; 1.2420x over previous
"""BiGRU encoder kernel for 8 Trainium2 NeuronCores (fp16, exact ragged schedule).

Strategy:
  - Masked fixed-position reformulation: forward runs positions ascending into
    the center, backward descending into the center, so every sample's
    recurrence ENDS on the final step.  A sample of length l starts updating
    at the step where need == l; before that its hidden state is held at 0
    exactly by forcing z = 1 (+BIG on the z pre-activation).
  - Samples sorted by window_len, dealt round-robin to 8 cores (data
    parallel).  Each core holds ONE sorted batch of 1024 columns
    (features on SBUF partitions, samples on the free dim).  Step j runs on
    the exact suffix W_j = max over cores of #samples with len >= need --
    fp16 matmuls are full rate at any width, so no minimum-width padding.
  - Within a step, columns split into F (samples taking their first step:
    h == 0) and R (already running).  The hidden projection runs ONLY on R;
    F columns take a cheap h'=(1-z)n update that never reads h_prev.
    Cross-core width slack is fixed up by a narrow mask strip on z.
  - The suffix splits into 512-wide groups (PSUM bank limit).  Gate biases
    are folded into scalar_tensor_tensor ops so tanh and the h-update run as
    single wide ops over [128, 4, W].
  - Output is written feature-major (yT) and transposed on the host.
"""

import os
from contextlib import ExitStack

import numpy as np

import concourse.bacc as bacc
import concourse.tile as tile
from concourse import mybir
from concourse.bass_utils import run_bass_kernel_spmd

NCORES = 8
B, T, D, H = 8192, 15, 512, 512
G = 3 * H
Bc = B // NCORES  # 1024 columns per core
BIG = 40.0
F32 = mybir.dt.float32
DT = mybir.dt.float16

ACT = mybir.ActivationFunctionType
ALU = mybir.AluOpType

_PROGRAM_CACHE = {}
LAST_RESULT = None


def _ceil(a, b):
    return -(-a // b)


def _build_program(sched):
    """sched = (f_steps, b_steps); each steps = tuple of (W, strip) per step,
    W monotone nondecreasing, W[-1] == Bc."""
    f_steps, b_steps = sched
    nf, nb = len(f_steps), len(b_steps)

    nc = bacc.Bacc("TRN2", target_bir_lowering=False, debug=False,
                   num_devices=NCORES)

    xT_d = nc.dram_tensor("xT", [T, D, Bc], DT, kind="ExternalInput")
    wf_d = nc.dram_tensor("wf", [D + H, G], DT, kind="ExternalInput")
    wb_d = nc.dram_tensor("wb", [D + H, G], DT, kind="ExternalInput")
    w1_d = nc.dram_tensor("w1", [2 * H, H], DT, kind="ExternalInput")
    w2_d = nc.dram_tensor("w2", [H, H], DT, kind="ExternalInput")
    bias_d = nc.dram_tensor("bias", [40, 128], F32, kind="ExternalInput")
    mf_d = nc.dram_tensor("maskzf", [nf, Bc], F32, kind="ExternalInput")
    mb_d = nc.dram_tensor("maskzb", [nb, Bc], F32, kind="ExternalInput")
    y_d = nc.dram_tensor("yT", [H, Bc], F32, kind="ExternalOutput")

    with tile.TileContext(nc) as tc, ExitStack() as ctx:
        const = ctx.enter_context(tc.tile_pool(name="const", bufs=1))
        wpool = ctx.enter_context(tc.tile_pool(name="w", bufs=2))
        xfp = ctx.enter_context(tc.tile_pool(name="xf", bufs=3))
        xbp = ctx.enter_context(tc.tile_pool(name="xb", bufs=2))
        hpool = ctx.enter_context(tc.tile_pool(name="h", bufs=2))
        hfin = ctx.enter_context(tc.tile_pool(name="hfin", bufs=2))
        rz4p = ctx.enter_context(tc.tile_pool(name="rz4", bufs=2))
        ssp = ctx.enter_context(tc.tile_pool(name="ss", bufs=2))
        np_ = ctx.enter_context(tc.tile_pool(name="n4", bufs=2))
        scr = ctx.enter_context(tc.tile_pool(name="scr", bufs=2))
        ttp = ctx.enter_context(tc.tile_pool(name="tt", bufs=4))
        obuf = ctx.enter_context(tc.tile_pool(name="o", bufs=2))
        mpool = ctx.enter_context(tc.tile_pool(name="m", bufs=2))
        accp = ctx.enter_context(tc.tile_pool(name="mlp", bufs=2))
        rzps = ctx.enter_context(tc.tile_pool(name="rz", bufs=2, space="PSUM"))
        xpps = ctx.enter_context(tc.tile_pool(name="xp", bufs=2, space="PSUM"))
        ghps = ctx.enter_context(tc.tile_pool(name="gh", bufs=2, space="PSUM"))

        # Weights as [128, kchunk, gate-cols]; kchunks 0-3 input dims, 4-7
        # hidden dims.  Per-kchunk DMAs so the first matmuls start as soon
        # as chunk 0 lands.
        def load_w(dram, kchunks, cols, name, pool, tag, eng):
            t_ = pool.tile([128, kchunks, cols], DT, tag=tag, name=name)
            src = dram.rearrange("(c k) g -> k c g", k=128)
            for c in range(kchunks):
                eng.dma_start(t_[:, c, :], src[:, c, :])
            return t_

        bt = const.tile([128, 40], F32)
        nc.gpsimd.dma_start(bt[:], bias_d.rearrange("n p -> p n"))
        wf = load_w(wf_d, 8, G, "wf", wpool, "w", nc.scalar)
        wb = load_w(wb_d, 8, G, "wb", wpool, "w", nc.scalar)
        w1 = load_w(w1_d, 8, H, "w1", const, "const", nc.gpsimd)
        w2 = load_w(w2_d, 4, H, "w2", const, "const", nc.gpsimd)

        def emit_x(steps, pool, tag, j, pos):
            W = steps[j][0]
            xt = pool.tile([128, 4, Bc], DT, tag=tag, name=f"{tag}{j}")
            nc.sync.dma_start(
                xt[:, :, :W],
                xT_d[pos].rearrange("(c k) s -> k c s", k=128)[:, :, Bc - W:],
            )
            return xt

        def emit_step(j, steps, xt, h_prev, w, mask_d, bb, is_last):
            """One GRU step.  Local cols 0..W-1 map to global Bc-W..Bc-1.
            F = [0, Fw): first-step columns.  R = [Fw, W): running."""
            W, strip = steps[j]
            Wprev = steps[j - 1][0] if j > 0 else 0
            Fw = W - Wprev
            goff = Bc - W  # local -> global

            h_next = (hfin if is_last else hpool).tile(
                [128, 4, Bc], DT, tag="hfin" if is_last else "h", name="h")

            mt = None
            if strip > 0:
                mt = mpool.tile([128, 256], F32, tag="m", name="mt")
                nc.gpsimd.dma_start(
                    mt[:, :strip],
                    mask_d[j, goff:goff + strip].partition_broadcast(128),
                )

            ngroups = _ceil(W, 512)
            for g in range(ngroups):  # left-aligned groups on local coords
                glo, ghi = 512 * g, min(512 * (g + 1), W)
                gw = ghi - glo
                fl, fh = glo, max(min(ghi, Fw), glo)   # F within group
                rl, rh = max(glo, Fw), ghi             # R within group
                fw, rw = fh - fl, rh - rl
                sl, sh = glo, max(min(ghi, strip), glo)  # mask strip in group
                sw = sh - sl

                rz = []
                xpn = []
                ghn = []
                for i in range(4):
                    ro, zo, no = i * 128, H + i * 128, 2 * H + i * 128
                    rzt = rzps.tile([128, 2, 512], F32, tag="rz", name=f"rz{i}")
                    xpt = xpps.tile([128, 512], F32, tag="xp", name=f"xp{i}")
                    rz.append(rzt)
                    xpn.append(xpt)
                    for k in range(4):
                        st = k == 0
                        xk = xt[:, k, glo:ghi]
                        if fw > 0 and rw > 0:
                            # F: start opens the bank (lazy-zeroes it); the
                            # R x-proj piggybacks with start=False and gets
                            # zero-init from the pending-zero region.  The
                            # h-proj's stop closes the bank group.
                            xkF = xt[:, k, fl:fh]
                            xkR = xt[:, k, rl:rh]
                            nc.tensor.matmul(rzt[:, 0, :fw], w[:, k, ro:ro + 128],
                                             xkF, start=st, stop=False)
                            nc.tensor.matmul(rzt[:, 0, fw:gw], w[:, k, ro:ro + 128],
                                             xkR, start=False, stop=False,
                                             skip_group_check=True)
                            nc.tensor.matmul(rzt[:, 1, :fw], w[:, k, zo:zo + 128],
                                             xkF, start=st, stop=False)
                            nc.tensor.matmul(rzt[:, 1, fw:gw], w[:, k, zo:zo + 128],
                                             xkR, start=False, stop=False,
                                             skip_group_check=True)
                        else:
                            sp = k == 3 and rw == 0
                            nc.tensor.matmul(rzt[:, 0, :gw], w[:, k, ro:ro + 128],
                                             xk, start=st, stop=sp)
                            nc.tensor.matmul(rzt[:, 1, :gw], w[:, k, zo:zo + 128],
                                             xk, start=st, stop=sp)
                        nc.tensor.matmul(xpt[:, :gw], w[:, k, no:no + 128],
                                         xk, start=st, stop=k == 3)
                    if rw > 0:
                        ght = ghps.tile([128, 512], F32, tag="gh", name=f"gh{i}")
                        ghn.append(ght)
                        fo = rl - glo  # local-in-group offset of R
                        for k in range(4):
                            hk = h_prev[:, k, goff + rl:goff + rh]
                            nc.tensor.matmul(rzt[:, 0, fo:gw],
                                             w[:, 4 + k, ro:ro + 128], hk,
                                             start=False, stop=k == 3)
                            nc.tensor.matmul(rzt[:, 1, fo:gw],
                                             w[:, 4 + k, zo:zo + 128], hk,
                                             start=False, stop=k == 3)
                            nc.tensor.matmul(ght[:, :rw],
                                             w[:, 4 + k, no:no + 128], hk,
                                             start=k == 0, stop=k == 3)

                # --- elementwise chain for this group ---
                rz4 = rz4p.tile([128, 2, 4, 512], DT, tag="rz4", name="rz4")
                ss4 = ssp.tile([128, 4, 512], DT, tag="ss", name="ss4")
                n4 = np_.tile([128, 4, 512], DT, tag="n4", name="n4")
                sc = scr.tile([128, 4, 512], DT, tag="scr", name="sc")

                for i in range(4):
                    # r = sigmoid(rps + bias_r)
                    nc.scalar.activation(rz4[:, 0, i, :gw], rz[i][:, 0, :gw],
                                         ACT.Sigmoid, bias=bt[:, bb + i:bb + i + 1])
                    # mask strip: force z -> 1 on over-included columns
                    if sw > 0:
                        nc.vector.tensor_add(rz[i][:, 1, sl - glo:sh - glo],
                                             rz[i][:, 1, sl - glo:sh - glo],
                                             mt[:, sl:sh])
                    nc.scalar.activation(rz4[:, 1, i, :gw], rz[i][:, 1, :gw],
                                         ACT.Sigmoid,
                                         bias=bt[:, bb + 4 + i:bb + 5 + i])
                    # n pre-activation: ss = xpn + bih_n + r * (ghn + bhh_n)
                    if fw > 0:
                        t1 = ttp.tile([128, 512], DT, tag="tt", name="t1")
                        nc.vector.tensor_scalar(
                            t1[:, :fw], rz4[:, 0, i, fl - glo:fh - glo],
                            bt[:, bb + 8 + i:bb + 9 + i],
                            bt[:, bb + 12 + i:bb + 13 + i],
                            op0=ALU.mult, op1=ALU.add)
                        nc.vector.tensor_add(
                            ss4[:, i, fl - glo:fh - glo], t1[:, :fw],
                            xpn[i][:, fl - glo:fh - glo])
                    if rw > 0:
                        t2 = ttp.tile([128, 512], DT, tag="tt", name="t2")
                        nc.vector.scalar_tensor_tensor(
                            t2[:, :rw], ghn[i][:, :rw],
                            bt[:, bb + 8 + i:bb + 9 + i],
                            rz4[:, 0, i, rl - glo:rh - glo],
                            op0=ALU.add, op1=ALU.mult)
                        nc.vector.scalar_tensor_tensor(
                            ss4[:, i, rl - glo:rh - glo], t2[:, :rw],
                            bt[:, bb + 12 + i:bb + 13 + i],
                            xpn[i][:, rl - glo:rh - glo],
                            op0=ALU.add, op1=ALU.add)

                # n = tanh(ss) over all 4 chunks at once
                nc.scalar.activation(n4[:, :, :gw], ss4[:, :, :gw], ACT.Tanh)

                # h update
                if fw > 0:
                    zF = rz4[:, 1, :, fl - glo:fh - glo]
                    nF = n4[:, :, fl - glo:fh - glo]
                    nc.vector.tensor_mul(sc[:, :, fl - glo:fh - glo], zF, nF)
                    nc.vector.tensor_sub(h_next[:, :, goff + fl:goff + fh],
                                         nF, sc[:, :, fl - glo:fh - glo])
                if rw > 0:
                    zR = rz4[:, 1, :, rl - glo:rh - glo]
                    nR = n4[:, :, rl - glo:rh - glo]
                    dd = sc[:, :, rl - glo:rh - glo]
                    nc.vector.tensor_sub(dd, h_prev[:, :, goff + rl:goff + rh],
                                         nR)
                    nc.vector.tensor_mul(dd, zR, dd)
                    nc.vector.tensor_add(h_next[:, :, goff + rl:goff + rh],
                                         nR, dd)
            return h_next

        def emit_dir(steps, w, mask_d, bb, pool, tag, pos_fn):
            n = len(steps)
            h = None
            for j in range(n):
                xt = emit_x(steps, pool, tag, j, pos_fn(j))
                h = emit_step(j, steps, xt, h, w, mask_d, bb, j == n - 1)
            return h

        hf4 = emit_dir(f_steps, wf, mf_d, 0, xfp, "xf",
                       lambda j: 8 - nf + j)

        # MLP phase A: acc = W1[:, :H].T @ hf  (runs while backward GRU owns
        # the critical path; result parked in SBUF)
        acc = accp.tile([128, 4, Bc], DT, tag="mlp", name="acc")
        for g in range(Bc // 512):
            for i in range(4):
                ps = xpps.tile([128, 512], F32, tag="xp", name="mlpA")
                for k in range(4):
                    nc.tensor.matmul(ps[:], w1[:, k, i * 128:(i + 1) * 128],
                                     hf4[:, k, g * 512:(g + 1) * 512],
                                     start=k == 0, stop=k == 3)
                nc.scalar.activation(acc[:, i, g * 512:(g + 1) * 512], ps[:],
                                     ACT.Copy)

        hb4 = emit_dir(b_steps, wb, mb_d, 16, xbp, "xb",
                       lambda j: 6 + nb - j)

        # MLP phase B: hid = relu(acc + W1[:, H:].T @ hb + b1)
        hid = accp.tile([128, 4, Bc], DT, tag="mlp", name="hid")
        for g in range(Bc // 512):
            pre = ssp.tile([128, 4, 512], DT, tag="ss", name="pre")
            for i in range(4):
                ps = xpps.tile([128, 512], F32, tag="xp", name="mlpB")
                for k in range(4):
                    nc.tensor.matmul(ps[:], w1[:, 4 + k, i * 128:(i + 1) * 128],
                                     hb4[:, k, g * 512:(g + 1) * 512],
                                     start=k == 0, stop=k == 3)
                nc.vector.scalar_tensor_tensor(
                    pre[:, i, :], ps[:], bt[:, 32 + i:33 + i],
                    acc[:, i, g * 512:(g + 1) * 512], op0=ALU.add, op1=ALU.add)
            nc.scalar.activation(hid[:, :, g * 512:(g + 1) * 512], pre[:],
                                 ACT.Relu)

        # MLP phase C: y = W2.T @ hid + b2, written feature-major
        for g in range(Bc // 512):
            for i in range(4):
                ps = xpps.tile([128, 512], F32, tag="xp", name="mlpC")
                for k in range(4):
                    nc.tensor.matmul(ps[:], w2[:, k, i * 128:(i + 1) * 128],
                                     hid[:, k, g * 512:(g + 1) * 512],
                                     start=k == 0, stop=k == 3)
                o32 = obuf.tile([128, 512], F32, tag="o", name="o32")
                nc.scalar.activation(o32[:], ps[:], ACT.Identity,
                                     bias=bt[:, 36 + i:37 + i])
                nc.scalar.dma_start(
                    y_d[i * 128:(i + 1) * 128, g * 512:(g + 1) * 512], o32[:])

    nc.compile()
    return nc


def kernel(padded_window, window_len, Wih_f, Whh_f, bih_f, bhh_f,
           Wih_b, Whh_b, bih_b, bhh_b, W1, b1, W2, b2):
    wl = np.asarray(window_len)
    lf = (wl - 1) // 2 + 1
    lb = wl // 2 + 1
    order = np.argsort(wl, kind="stable")

    # per-core sorted lengths: row k = per-core rank k, column = core
    lf_pc = lf[order].reshape(-1, NCORES)
    lb_pc = lb[order].reshape(-1, NCORES)

    def dir_steps(lens_pc):
        n = int(lens_pc.max())
        steps, cnts = [], []
        for j in range(n):
            need = n - j
            cnt = (lens_pc >= need).sum(axis=0)  # per core
            W = int(cnt.max())
            strip = W - int(cnt.min())
            steps.append((W, strip))
            cnts.append(cnt)
        return tuple(steps), cnts

    f_steps, f_cnts = dir_steps(lf_pc)
    b_steps, b_cnts = dir_steps(lb_pc)
    sched = (f_steps, b_steps)

    if sched not in _PROGRAM_CACHE:
        _PROGRAM_CACHE[sched] = _build_program(sched)
    nc = _PROGRAM_CACHE[sched]

    f16 = np.float16
    f32 = np.float32
    wf = np.concatenate([Wih_f.T, Whh_f.T], 0).astype(f16)
    wb = np.concatenate([Wih_b.T, Whh_b.T], 0).astype(f16)
    w1 = np.ascontiguousarray(W1.T, dtype=f16)
    w2 = np.ascontiguousarray(W2.T, dtype=f16)

    def chunks(v):  # [512] -> [4, 128]
        return np.asarray(v, f32).reshape(4, 128)

    bias = np.concatenate([
        chunks((bih_f + bhh_f)[:H]), chunks((bih_f + bhh_f)[H:2 * H]),
        chunks(bhh_f[2 * H:]), chunks(bih_f[2 * H:]),
        chunks((bih_b + bhh_b)[:H]), chunks((bih_b + bhh_b)[H:2 * H]),
        chunks(bhh_b[2 * H:]), chunks(bih_b[2 * H:]),
        chunks(b1), chunks(b2),
    ], 0)  # [40, 128]

    pw16 = np.asarray(padded_window).astype(f16)
    pos = np.arange(Bc)
    in_maps = []
    for c in range(NCORES):
        idx = order[c::NCORES]
        xT = np.ascontiguousarray(pw16[idx].transpose(1, 2, 0))  # [15,512,Bc]
        mzf = np.stack([(BIG * (pos < Bc - cnt[c])).astype(f32)
                        for cnt in f_cnts])
        mzb = np.stack([(BIG * (pos < Bc - cnt[c])).astype(f32)
                        for cnt in b_cnts])
        in_maps.append({
            "xT": xT, "wf": wf, "wb": wb, "w1": w1, "w2": w2,
            "bias": bias, "maskzf": mzf, "maskzb": mzb,
        })

    trace = bool(os.environ.get("GRU_TRACE"))
    kw = {}
    if os.environ.get("GRU_TMPDIR"):
        kw["tmpdir"] = os.environ["GRU_TMPDIR"]
    res = run_bass_kernel_spmd(nc, in_maps, core_ids=list(range(NCORES)),
                               trace=trace, **kw)
    global LAST_RESULT
    LAST_RESULT = res
    out = np.empty((B, H), f32)
    for c in range(NCORES):
        out[order[c::NCORES]] = res.results[c]["yT"].T
    return out


# revision 13
# speedup vs baseline: 1.2914x; 1.0397x over previous
"""BiGRU encoder kernel for 8 Trainium2 NeuronCores (fp16, exact ragged schedule).

Strategy:
  - Masked fixed-position reformulation: forward runs positions ascending into
    the center, backward descending into the center, so every sample's
    recurrence ENDS on the final step.  A sample of length l starts updating
    at the step where need == l; before that its hidden state is held at 0
    exactly by forcing z = 1 (+BIG on the z pre-activation).
  - Samples sorted by window_len, dealt round-robin to 8 cores (data
    parallel).  Each core holds ONE sorted batch of 1024 columns
    (features on SBUF partitions, samples on the free dim).  Step j runs on
    the exact suffix W_j = max over cores of #samples with len >= need --
    fp16 matmuls are full rate at any width, so no minimum-width padding.
  - Within a step, columns split into F (samples taking their first step:
    h == 0) and R (already running).  The hidden projection runs ONLY on R;
    F columns take a cheap h'=(1-z)n update that never reads h_prev.
    Cross-core width slack is fixed up by a narrow mask strip on z.
  - The suffix splits into 512-wide groups (PSUM bank limit).  Gate biases
    are folded into scalar_tensor_tensor ops so tanh and the h-update run as
    single wide ops over [128, 4, W].
  - Output is written feature-major (yT) and transposed on the host.
"""

import os
from contextlib import ExitStack

import numpy as np

import concourse.bacc as bacc
import concourse.tile as tile
from concourse import mybir
from concourse.bass_utils import run_bass_kernel_spmd

NCORES = 8
B, T, D, H = 8192, 15, 512, 512
G = 3 * H
Bc = B // NCORES  # 1024 columns per core
BIG = 40.0
F32 = mybir.dt.float32
DT = mybir.dt.float16

ACT = mybir.ActivationFunctionType
ALU = mybir.AluOpType

_PROGRAM_CACHE = {}
LAST_RESULT = None


def _ceil(a, b):
    return -(-a // b)


def _build_program(sched):
    """sched = (f_steps, b_steps); each steps = tuple of (W, strip) per step,
    W monotone nondecreasing, W[-1] == Bc."""
    f_steps, b_steps = sched
    nf, nb = len(f_steps), len(b_steps)

    nc = bacc.Bacc("TRN2", target_bir_lowering=False, debug=False,
                   num_devices=NCORES)

    S4 = 4 * (sum(w for w, _ in f_steps) + sum(w for w, _ in b_steps))
    xpk_d = nc.dram_tensor("xpk", [128, S4], DT, kind="ExternalInput")
    wf_d = nc.dram_tensor("wf", [D + H, G], DT, kind="ExternalInput")
    wb_d = nc.dram_tensor("wb", [D + H, G], DT, kind="ExternalInput")
    w1_d = nc.dram_tensor("w1", [2 * H, H], DT, kind="ExternalInput")
    w2_d = nc.dram_tensor("w2", [H, H], DT, kind="ExternalInput")
    bias_d = nc.dram_tensor("bias", [40, 128], F32, kind="ExternalInput")
    mf_d = nc.dram_tensor("maskzf", [nf, Bc], F32, kind="ExternalInput")
    mb_d = nc.dram_tensor("maskzb", [nb, Bc], F32, kind="ExternalInput")
    y_d = nc.dram_tensor("yT", [H, Bc], F32, kind="ExternalOutput")

    with tile.TileContext(nc) as tc, ExitStack() as ctx:
        const = ctx.enter_context(tc.tile_pool(name="const", bufs=1))
        wpool = ctx.enter_context(tc.tile_pool(name="w", bufs=2))
        xfp = ctx.enter_context(tc.tile_pool(name="xf", bufs=3))
        xbp = ctx.enter_context(tc.tile_pool(name="xb", bufs=2))
        hpool = ctx.enter_context(tc.tile_pool(name="h", bufs=2))
        hfin = ctx.enter_context(tc.tile_pool(name="hfin", bufs=2))
        rz4p = ctx.enter_context(tc.tile_pool(name="rz4", bufs=2))
        ssp = ctx.enter_context(tc.tile_pool(name="ss", bufs=2))
        np_ = ctx.enter_context(tc.tile_pool(name="n4", bufs=2))
        scr = ctx.enter_context(tc.tile_pool(name="scr", bufs=2))
        ttp = ctx.enter_context(tc.tile_pool(name="tt", bufs=4))
        obuf = ctx.enter_context(tc.tile_pool(name="o", bufs=2))
        mpool = ctx.enter_context(tc.tile_pool(name="m", bufs=2))
        accp = ctx.enter_context(tc.tile_pool(name="mlp", bufs=2))
        rzps = ctx.enter_context(tc.tile_pool(name="rz", bufs=2, space="PSUM"))
        xpps = ctx.enter_context(tc.tile_pool(name="xp", bufs=2, space="PSUM"))
        ghps = ctx.enter_context(tc.tile_pool(name="gh", bufs=2, space="PSUM"))

        # Weights as [128, kchunk, gate-cols]; kchunks 0-3 input dims, 4-7
        # hidden dims.  Per-kchunk DMAs so the first matmuls start as soon
        # as chunk 0 lands.
        def load_w(dram, kchunks, cols, name, pool, tag, eng):
            t_ = pool.tile([128, kchunks, cols], DT, tag=tag, name=name)
            src = dram.rearrange("(c k) g -> k c g", k=128)
            for c in range(kchunks):
                eng.dma_start(t_[:, c, :], src[:, c, :])
            return t_

        bt = const.tile([128, 40], F32)
        nc.gpsimd.dma_start(bt[:], bias_d.rearrange("n p -> p n"))
        wf = load_w(wf_d, 8, G, "wf", wpool, "w", nc.scalar)
        wb = load_w(wb_d, 8, G, "wb", wpool, "w", nc.gpsimd)
        w1 = load_w(w1_d, 8, H, "w1", const, "w1", nc.gpsimd)
        w2 = load_w(w2_d, 4, H, "w2", const, "w2", nc.gpsimd)

        def emit_x(steps, pool, tag, j, off):
            W = steps[j][0]
            xt = pool.tile([128, 4, Bc], DT, tag=tag, name=f"{tag}{j}")
            nc.sync.dma_start(
                xt[:, :, :W],
                xpk_d[:, off:off + 4 * W].rearrange("p (k w) -> p k w", k=4),
            )
            return xt

        def emit_step(j, steps, xt, h_prev, w, mask_d, bb, is_last):
            """One GRU step.  Local cols 0..W-1 map to global Bc-W..Bc-1.
            F = [0, Fw): first-step columns.  R = [Fw, W): running."""
            W, strip = steps[j]
            Wprev = steps[j - 1][0] if j > 0 else 0
            Fw = W - Wprev
            goff = Bc - W  # local -> global

            h_next = (hfin if is_last else hpool).tile(
                [128, 4, Bc], DT, tag="hfin" if is_last else "h", name="h")

            mt = None
            if strip > 0:
                mt = mpool.tile([128, 256], F32, tag="m", name="mt")
                nc.sync.dma_start(
                    mt[:, :strip],
                    mask_d[j, goff:goff + strip].partition_broadcast(128),
                )

            ngroups = _ceil(W, 512)
            for g in range(ngroups):  # left-aligned groups on local coords
                glo, ghi = 512 * g, min(512 * (g + 1), W)
                gw = ghi - glo
                fl, fh = glo, max(min(ghi, Fw), glo)   # F within group
                rl, rh = max(glo, Fw), ghi             # R within group
                fw, rw = fh - fl, rh - rl
                sl, sh = glo, max(min(ghi, strip), glo)  # mask strip in group
                sw = sh - sl

                rz = []
                xpn = []
                ghn = []
                for i in range(4):
                    ro, zo, no = i * 128, H + i * 128, 2 * H + i * 128
                    rzt = rzps.tile([128, 2, 512], F32, tag="rz", name=f"rz{i}")
                    xpt = xpps.tile([128, 512], F32, tag="xp", name=f"xp{i}")
                    rz.append(rzt)
                    xpn.append(xpt)
                    for k in range(4):
                        st = k == 0
                        xk = xt[:, k, glo:ghi]
                        if fw > 0 and rw > 0:
                            # F: start opens the bank (lazy-zeroes it); the
                            # R x-proj piggybacks with start=False and gets
                            # zero-init from the pending-zero region.  The
                            # h-proj's stop closes the bank group.
                            xkF = xt[:, k, fl:fh]
                            xkR = xt[:, k, rl:rh]
                            nc.tensor.matmul(rzt[:, 0, :fw], w[:, k, ro:ro + 128],
                                             xkF, start=st, stop=False)
                            nc.tensor.matmul(rzt[:, 0, fw:gw], w[:, k, ro:ro + 128],
                                             xkR, start=False, stop=False,
                                             skip_group_check=True)
                            nc.tensor.matmul(rzt[:, 1, :fw], w[:, k, zo:zo + 128],
                                             xkF, start=st, stop=False)
                            nc.tensor.matmul(rzt[:, 1, fw:gw], w[:, k, zo:zo + 128],
                                             xkR, start=False, stop=False,
                                             skip_group_check=True)
                        else:
                            sp = k == 3 and rw == 0
                            nc.tensor.matmul(rzt[:, 0, :gw], w[:, k, ro:ro + 128],
                                             xk, start=st, stop=sp)
                            nc.tensor.matmul(rzt[:, 1, :gw], w[:, k, zo:zo + 128],
                                             xk, start=st, stop=sp)
                        nc.tensor.matmul(xpt[:, :gw], w[:, k, no:no + 128],
                                         xk, start=st, stop=k == 3)
                    if rw > 0:
                        ght = ghps.tile([128, 512], F32, tag="gh", name=f"gh{i}")
                        ghn.append(ght)
                        fo = rl - glo  # local-in-group offset of R
                        for k in range(4):
                            hk = h_prev[:, k, goff + rl:goff + rh]
                            nc.tensor.matmul(rzt[:, 0, fo:gw],
                                             w[:, 4 + k, ro:ro + 128], hk,
                                             start=False, stop=k == 3)
                            nc.tensor.matmul(rzt[:, 1, fo:gw],
                                             w[:, 4 + k, zo:zo + 128], hk,
                                             start=False, stop=k == 3)
                            nc.tensor.matmul(ght[:, :rw],
                                             w[:, 4 + k, no:no + 128], hk,
                                             start=k == 0, stop=k == 3)

                # --- elementwise chain for this group, per i-pair so the
                # next step's h-projection k-chunks can start early ---
                rz4 = rz4p.tile([128, 2, 4, 512], DT, tag="rz4", name="rz4")
                ss4 = ssp.tile([128, 4, 512], DT, tag="ss", name="ss4")
                n4 = np_.tile([128, 4, 512], DT, tag="n4", name="n4")
                sc = scr.tile([128, 4, 512], DT, tag="scr", name="sc")

                def upd_pair(p0):
                    ii = slice(p0, p0 + 2)
                    nc.scalar.activation(n4[:, ii, :gw], ss4[:, ii, :gw],
                                         ACT.Tanh)
                    if fw > 0:
                        zF = rz4[:, 1, ii, fl - glo:fh - glo]
                        nF = n4[:, ii, fl - glo:fh - glo]
                        eF = sc[:, ii, fl - glo:fh - glo]
                        nc.vector.tensor_mul(eF, zF, nF)
                        nc.vector.tensor_sub(h_next[:, ii, goff + fl:goff + fh],
                                             nF, eF)
                    if rw > 0:
                        zR = rz4[:, 1, ii, rl - glo:rh - glo]
                        nR = n4[:, ii, rl - glo:rh - glo]
                        dd = sc[:, ii, rl - glo:rh - glo]
                        nc.vector.tensor_sub(dd, h_prev[:, ii, goff + rl:goff + rh],
                                             nR)
                        nc.vector.tensor_mul(dd, zR, dd)
                        nc.vector.tensor_add(h_next[:, ii, goff + rl:goff + rh],
                                             nR, dd)

                for i in range(4):
                    # r = sigmoid(rps + bias_r)
                    nc.scalar.activation(rz4[:, 0, i, :gw], rz[i][:, 0, :gw],
                                         ACT.Sigmoid, bias=bt[:, bb + i:bb + i + 1])
                    # mask strip: force z -> 1 on over-included columns
                    if sw > 0:
                        nc.vector.tensor_add(rz[i][:, 1, sl - glo:sh - glo],
                                             rz[i][:, 1, sl - glo:sh - glo],
                                             mt[:, sl:sh])
                    nc.scalar.activation(rz4[:, 1, i, :gw], rz[i][:, 1, :gw],
                                         ACT.Sigmoid,
                                         bias=bt[:, bb + 4 + i:bb + 5 + i])
                    # n pre-activation: ss = xpn + bih_n + r * (ghn + bhh_n)
                    if fw > 0:
                        t1 = ttp.tile([128, 512], DT, tag="tt", name="t1")
                        nc.vector.tensor_scalar(
                            t1[:, :fw], rz4[:, 0, i, fl - glo:fh - glo],
                            bt[:, bb + 8 + i:bb + 9 + i],
                            bt[:, bb + 12 + i:bb + 13 + i],
                            op0=ALU.mult, op1=ALU.add)
                        nc.vector.tensor_add(
                            ss4[:, i, fl - glo:fh - glo], t1[:, :fw],
                            xpn[i][:, fl - glo:fh - glo])
                    if rw > 0:
                        t2 = ttp.tile([128, 512], DT, tag="tt", name="t2")
                        nc.vector.scalar_tensor_tensor(
                            t2[:, :rw], ghn[i][:, :rw],
                            bt[:, bb + 8 + i:bb + 9 + i],
                            rz4[:, 0, i, rl - glo:rh - glo],
                            op0=ALU.add, op1=ALU.mult)
                        nc.vector.scalar_tensor_tensor(
                            ss4[:, i, rl - glo:rh - glo], t2[:, :rw],
                            bt[:, bb + 12 + i:bb + 13 + i],
                            xpn[i][:, rl - glo:rh - glo],
                            op0=ALU.add, op1=ALU.add)
                    if i == 1:
                        upd_pair(0)
                    elif i == 3:
                        upd_pair(2)
            return h_next

        def emit_dir(steps, w, mask_d, bb, pool, tag, off0):
            n = len(steps)
            h = None
            off = off0
            for j in range(n):
                xt = emit_x(steps, pool, tag, j, off)
                off += 4 * steps[j][0]
                h = emit_step(j, steps, xt, h, w, mask_d, bb, j == n - 1)
            return h

        hf4 = emit_dir(f_steps, wf, mf_d, 0, xfp, "xf", 0)

        # MLP phase A: acc = W1[:, :H].T @ hf  (runs while backward GRU owns
        # the critical path; result parked in SBUF)
        acc = accp.tile([128, 4, Bc], DT, tag="mlp", name="acc")
        for g in range(Bc // 512):
            for i in range(4):
                ps = xpps.tile([128, 512], F32, tag="xp", name="mlpA")
                for k in range(4):
                    nc.tensor.matmul(ps[:], w1[:, k, i * 128:(i + 1) * 128],
                                     hf4[:, k, g * 512:(g + 1) * 512],
                                     start=k == 0, stop=k == 3)
                nc.scalar.activation(acc[:, i, g * 512:(g + 1) * 512], ps[:],
                                     ACT.Copy)

        hb4 = emit_dir(b_steps, wb, mb_d, 16, xbp, "xb",
                       4 * sum(w for w, _ in f_steps))

        # MLP phases B+C interleaved per column group:
        #   hid = relu(acc + W1[:, H:].T @ hb + b1);  y = W2.T @ hid + b2
        hid = accp.tile([128, 4, Bc], DT, tag="mlp", name="hid")
        for g in range(Bc // 512):
            gs = slice(g * 512, (g + 1) * 512)
            pre = ssp.tile([128, 4, 512], DT, tag="ss", name="pre")
            for i in range(4):
                ps = xpps.tile([128, 512], F32, tag="xp", name="mlpB")
                for k in range(4):
                    nc.tensor.matmul(ps[:], w1[:, 4 + k, i * 128:(i + 1) * 128],
                                     hb4[:, k, gs], start=k == 0, stop=k == 3)
                nc.vector.scalar_tensor_tensor(
                    pre[:, i, :], ps[:], bt[:, 32 + i:33 + i],
                    acc[:, i, gs], op0=ALU.add, op1=ALU.add)
            nc.scalar.activation(hid[:, :, gs], pre[:], ACT.Relu)
            for i in range(4):
                ps = xpps.tile([128, 512], F32, tag="xp", name="mlpC")
                for k in range(4):
                    nc.tensor.matmul(ps[:], w2[:, k, i * 128:(i + 1) * 128],
                                     hid[:, k, gs], start=k == 0, stop=k == 3)
                o32 = obuf.tile([128, 512], F32, tag="o", name="o32")
                nc.scalar.activation(o32[:], ps[:], ACT.Identity,
                                     bias=bt[:, 36 + i:37 + i])
                nc.scalar.dma_start(
                    y_d[i * 128:(i + 1) * 128, gs], o32[:])

    nc.compile()
    return nc


def kernel(padded_window, window_len, Wih_f, Whh_f, bih_f, bhh_f,
           Wih_b, Whh_b, bih_b, bhh_b, W1, b1, W2, b2):
    wl = np.asarray(window_len)
    lf = (wl - 1) // 2 + 1
    lb = wl // 2 + 1
    order = np.argsort(wl, kind="stable")

    # per-core sorted lengths: row k = per-core rank k, column = core
    lf_pc = lf[order].reshape(-1, NCORES)
    lb_pc = lb[order].reshape(-1, NCORES)

    def dir_steps(lens_pc):
        n = int(lens_pc.max())
        steps, cnts = [], []
        for j in range(n):
            need = n - j
            cnt = (lens_pc >= need).sum(axis=0)  # per core
            W = int(cnt.max())
            strip = W - int(cnt.min())
            assert strip <= 256, f"mask strip {strip} exceeds tile"
            steps.append((W, strip))
            cnts.append(cnt)
        return tuple(steps), cnts

    f_steps, f_cnts = dir_steps(lf_pc)
    b_steps, b_cnts = dir_steps(lb_pc)
    sched = (f_steps, b_steps)

    if sched not in _PROGRAM_CACHE:
        _PROGRAM_CACHE[sched] = _build_program(sched)
    nc = _PROGRAM_CACHE[sched]

    f16 = np.float16
    f32 = np.float32
    wf = np.concatenate([Wih_f.T, Whh_f.T], 0).astype(f16)
    wb = np.concatenate([Wih_b.T, Whh_b.T], 0).astype(f16)
    w1 = np.ascontiguousarray(W1.T, dtype=f16)
    w2 = np.ascontiguousarray(W2.T, dtype=f16)

    def chunks(v):  # [512] -> [4, 128]
        return np.asarray(v, f32).reshape(4, 128)

    bias = np.concatenate([
        chunks((bih_f + bhh_f)[:H]), chunks((bih_f + bhh_f)[H:2 * H]),
        chunks(bhh_f[2 * H:]), chunks(bih_f[2 * H:]),
        chunks((bih_b + bhh_b)[:H]), chunks((bih_b + bhh_b)[H:2 * H]),
        chunks(bhh_b[2 * H:]), chunks(bih_b[2 * H:]),
        chunks(b1), chunks(b2),
    ], 0)  # [40, 128]

    pw16 = np.asarray(padded_window).astype(f16)
    pos = np.arange(Bc)
    nf, nb = len(f_steps), len(b_steps)
    in_maps = []
    for c in range(NCORES):
        idx = order[c::NCORES]
        xTc = pw16[idx].transpose(1, 2, 0)  # [15, 512, Bc] (view-ish)
        blocks = []
        for steps, pfn in ((f_steps, lambda j: 8 - nf + j),
                           (b_steps, lambda j: 6 + nb - j)):
            for j, (W, _) in enumerate(steps):
                sl = xTc[pfn(j), :, Bc - W:]  # [512, W]
                blocks.append(sl.reshape(4, 128, W).transpose(1, 0, 2)
                              .reshape(128, 4 * W))
        xpk = np.ascontiguousarray(np.concatenate(blocks, axis=1))
        mzf = np.stack([(BIG * (pos < Bc - cnt[c])).astype(f32)
                        for cnt in f_cnts])
        mzb = np.stack([(BIG * (pos < Bc - cnt[c])).astype(f32)
                        for cnt in b_cnts])
        in_maps.append({
            "xpk": xpk, "wf": wf, "wb": wb, "w1": w1, "w2": w2,
            "bias": bias, "maskzf": mzf, "maskzb": mzb,
        })

    trace = bool(os.environ.get("GRU_TRACE"))
    kw = {}
    if os.environ.get("GRU_TMPDIR"):
        kw["tmpdir"] = os.environ["GRU_TMPDIR"]
    res = run_bass_kernel_spmd(nc, in_maps, core_ids=list(range(NCORES)),
                               trace=trace, **kw)
    global LAST_RESULT
    LAST_RESULT = res
    out = np.empty((B, H), f32)
    for c in range(NCORES):
        out[order[c::NCORES]] = res.results[c]["yT"].T
    return out
